# revision 1
# baseline (speedup 1.0000x reference)
"""Trainium2 Bass kernel for nn_Interpolator: zero-stuff upsample x8 + 128-tap FIR (SAME) + x8 gain.

Polyphase formulation: with m indexing 64-sample rows of x and n = 8*q' + r in [0, 512),
    y[512*m + n] = sum_{k=0}^{78} T4[k, m] * H4[k, n]
where T4[k, m] = x[64*m + k - 7] (zero-padded) and
    H4[k, 8*q'+r] = 8 * h[(7-r) + 8*(k-q')]  for 0 <= k-q' <= 15, else 0.

Per core (8 cores, batch-parallel): 16 signals (8 batch rows x {real, imag}).
Because NPAD = 64*513, one xbar DMA-transpose with src rows of stride 64 spanning a
whole GROUP of consecutive signals yields every signal's T4 at column offset 513*sig.
Merged transposes in GROUPS=(2,2,4,4,4) replace 16 small ones; the early groups
are small so the first signals' matmuls start earliest and stay ahead of the PE
even when ambient HBM noise stretches a transpose; later groups are big for xbar
efficiency.

Xbar-mode transitions serialize globally against plain DMAs (HWDGE *and* SWDGE)
with a multi-us drain, so the kernel front is a PURE xbar block on the sync queue:
H4 is host-transposed and loaded via xbar as well ([512,128] rows -> [128,512]
SBUF).  Stores (plain DMAs) only start after the block; they are copy-paced
anyway.  Concurrent xbar transposes from both HWDGE queues corrupt data — all
transposes stay on one queue.

The PE is pinned at 1.2 GHz on this part (10+ us of gapless matmuls never
unthrottle HAM), so matmul time is 512 cols / 1.2 GHz = 427 ns; the 64 matmuls
(27.5 us) run back-to-back with zero gaps and are the critical path.  PSUM is
tiled [128, 1024] (2 banks) x 4 bufs: vector casts the t0/t1 half while the PE
runs t2/t3, keeping the PSUM-recycle bubble off the critical path; scalar casts
the t2/t3 half (split across both engines for the last signal to shorten the
drain tail).  y is fp16 on device; the host casts to fp32.
"""

import numpy as np

import concourse.bass as bass
import concourse.tile as tile
from concourse import bacc, mybir
from concourse.bass_utils import run_bass_kernel_spmd

B = 64
N = 32768
FACTOR = 8
NOUT = N * FACTOR  # 262144
N_CORES = 8
ROWS_PER_CORE = B // N_CORES  # 8
SIGS = 2 * ROWS_PER_CORE  # 16 signals per core (real rows then imag rows)
K = 79  # contraction window length
NPAD = 32832  # 7 leading zeros + N + 57 trailing zeros; = 64*513
TILES = 4  # out tiles per signal, each [128 m-rows, 512 samples]

# Transpose groups: first group small so sig 0's matmuls start earliest; the
# rest merged big (feed rate only needs to stay ahead of the PE).
GROUPS = (2, 2, 4, 4, 4)  # signals per merged transpose


def _rows(n_sigs):
    """Src rows for an n-signal merged transpose, padded to the xbar multiple."""
    r = 513 * (n_sigs - 1) + 512
    return (r + 15) // 16 * 16


# src overrun past the end of a group's span (max over groups)
XTRA = max(64 * (_rows(g) - 1) + 128 - g * NPAD for g in GROUPS)

_F16 = mybir.dt.float16
_F32 = mybir.dt.float32

_NC_CACHE = {}


def _build_nc():
    nc = bacc.Bacc(
        "TRN2",
        target_bir_lowering=False,
        debug=False,
        enable_asserts=False,
        num_devices=N_CORES,
    )
    x = nc.dram_tensor("x", [SIGS * NPAD + XTRA], _F16, kind="ExternalInput")
    h4t = nc.dram_tensor("h4t", [512 * 128], _F16, kind="ExternalInput")
    y = nc.dram_tensor("y", [SIGS, NOUT], _F16, kind="ExternalOutput")

    with tile.TileContext(nc) as tc:
        with (
            tc.tile_pool(name="consts", bufs=1) as consts,
            tc.tile_pool(name="t4pool", bufs=len(GROUPS)) as t4pool,
            tc.tile_pool(name="opool", bufs=8) as opool,
            tc.tile_pool(name="po", bufs=4, space="PSUM") as po_pool,
        ):
            # H4 via xbar: h4_sb[k, c] = h4t[128c + k] = H4[k, c] (rows 79..127 zero)
            h4_sb = consts.tile([128, 512], _F16)
            nc.sync.dma_start(
                out=h4_sb[:, :],
                in_=bass.AP(tensor=h4t, offset=0, ap=[[128, 512], [1, 128]]),
                transpose=True,
            )

            # per-signal (tile, local col base) after its group's transpose
            t4_of_sig = [None] * SIGS

            def xpose(first_sig, n_sigs):
                """Merged xbar transpose: T4g[k, 513*s + m] = x_pad[first+s, 64m + k]."""
                rows = _rows(n_sigs)
                T4g = t4pool.tile([128, rows], _F16, tag="t4")
                nc.sync.dma_start(
                    out=T4g[:, :],
                    in_=bass.AP(
                        tensor=x,
                        offset=first_sig * NPAD,
                        ap=[[64, rows], [1, 128]],
                    ),
                    transpose=True,
                )
                for s in range(n_sigs):
                    t4_of_sig[first_sig + s] = (T4g, 513 * s)

            def compute_store(sig):
                """4 matmuls -> 2x [128,1024] PSUM, two half casts, one 512 KB store."""
                T4g, base = t4_of_sig[sig]
                out_sb = opool.tile([128, TILES * 512], _F16)
                for half in range(2):
                    po = po_pool.tile([128, 1024], _F32, tag="po")
                    for s in range(2):
                        t = 2 * half + s
                        nc.tensor.matmul(
                            po[:, 512 * s : 512 * (s + 1)],
                            T4g[0:K, base + 128 * t : base + 128 * (t + 1)],
                            h4_sb[0:K, :],
                            start=True,
                            stop=True,
                        )
                    sl = slice(1024 * half, 1024 * (half + 1))
                    if half == 0:
                        nc.vector.tensor_copy(out=out_sb[:, sl], in_=po[:, :])
                    elif sig == SIGS - 1:
                        # last signal: split the tail copy across both engines so
                        # the final store (and the kernel drain) starts sooner
                        nc.scalar.copy(out=out_sb[:, 1024:1536], in_=po[:, 0:512])
                        nc.vector.tensor_copy(
                            out=out_sb[:, 1536:2048], in_=po[:, 512:1024]
                        )
                    else:
                        nc.scalar.copy(out=out_sb[:, sl], in_=po[:, :])
                # partition i, free (t, n) -> y[sig, 65536t + 512i + n]
                if sig == SIGS - 1:
                    # last signal: two half stores so the first half lands while
                    # the tail copies run — the final drain starts ~1us sooner
                    for h in range(2):
                        nc.sync.dma_start(
                            out=bass.AP(
                                tensor=y,
                                offset=sig * NOUT + h * 2 * 65536,
                                ap=[[512, 128], [65536, 2], [1, 512]],
                            ),
                            in_=out_sb[:, 1024 * h : 1024 * (h + 1)],
                        )
                else:
                    nc.sync.dma_start(
                        out=bass.AP(
                            tensor=y,
                            offset=sig * NOUT,
                            ap=[[512, 128], [65536, TILES], [1, 512]],
                        ),
                        in_=out_sb[:, :],
                    )

            first = 0
            for g in GROUPS:
                xpose(first, g)
                first += g
            for sig in range(SIGS):
                compute_store(sig)

    nc.compile()
    return nc


def _get_nc():
    if "nc" not in _NC_CACHE:
        _NC_CACHE["nc"] = _build_nc()
    return _NC_CACHE["nc"]


def _build_h4(h):
    h4 = np.zeros((K, 512), np.float32)
    qp = np.arange(64)
    for t in range(16):
        for r in range(8):
            h4[qp + t, 8 * qp + r] = FACTOR * h[(7 - r) + 8 * t]
    return h4


def _run(x_real, x_imag, fir_filter, trace=False):
    h4 = _build_h4(np.asarray(fir_filter, np.float32)).astype(np.float16)
    h4t = np.zeros((512, 128), np.float16)
    h4t[:, :K] = h4.T
    in_maps = []
    for c in range(N_CORES):
        rows = slice(c * ROWS_PER_CORE, (c + 1) * ROWS_PER_CORE)
        shard = np.zeros((SIGS, NPAD), np.float16)
        shard[:ROWS_PER_CORE, 7 : 7 + N] = x_real[rows]
        shard[ROWS_PER_CORE:, 7 : 7 + N] = x_imag[rows]
        flat = np.zeros(SIGS * NPAD + XTRA, np.float16)
        flat[: SIGS * NPAD] = shard.reshape(-1)
        in_maps.append({"x": flat, "h4t": h4t.reshape(-1)})
    nc = _get_nc()
    res = run_bass_kernel_spmd(nc, in_maps, core_ids=list(range(N_CORES)), trace=trace)
    out = np.empty((2, B, NOUT), np.float32)
    for c in range(N_CORES):
        yc = res.results[c]["y"]
        rows = slice(c * ROWS_PER_CORE, (c + 1) * ROWS_PER_CORE)
        out[0, rows] = yc[:ROWS_PER_CORE]
        out[1, rows] = yc[ROWS_PER_CORE:]
    return out, res


def kernel(x_real, x_imag, fir_filter, factor):
    assert int(factor) == FACTOR
    x_real = np.asarray(x_real, np.float32)
    x_imag = np.asarray(x_imag, np.float32)
    assert x_real.shape == (B, N) and x_imag.shape == (B, N)
    out, _ = _run(x_real, x_imag, fir_filter)
    return out

